# revision 7
# baseline (speedup 1.0000x reference)
"""Trainium2 Bass kernel for nn_Decoder_36636071035490.

Reference computes, for workers i and task/edge (j,l):
    z = worker_feature @ W            # [2000, 1]
    p1 = sigmoid(z + b)
    p2 = (1 - p1) / 9
    P[i, j, l] = p1_i^tau_jl * p2_i^(1 - tau_jl)      # [2000, 5000, 10] f32

Identity used on device (exact in exact arithmetic):
    P[i, f] = exp(a_i * tau_f + c_i)
    a_i = (z_i + b) + ln 9            # since logit(sigmoid(x)) = x
    p2_i = 1 / (9 * (1 + exp(z_i + b)))
    P[i, f] = p2_i * exp(a_i * tau_f)

Sharding: by output columns (task*edge flattened, 50000 -> 8 x 6250); every
core computes the cheap per-worker scalars a/c for all 2000 workers
(replicated matvec) and produces the full-height [2000, 6250] slab of P.

Device-side layout (all tuned against the DMA-engine trace):
- tau is loaded ONCE as [1, 6250] (25 KB, one descriptor) and broadcast
  across the 128 SBUF partitions by the idle PE (ones[1,128]^T @ tau chunk
  -> PSUM bank, DVE copies PSUM->SBUF).  This replaces the 3.2 MB
  replicated HBM read of the previous version (~8 us of DMA-engine time).
- workers are processed in blocks of 256 rows, interleaved two-per-
  partition (partition p of block g holds workers 256g+2p and 256g+2p+1),
  so a block's store is one DMA whose per-partition descriptor is a
  CONTIGUOUS 50 KB range of the output - half the descriptor count of a
  row-per-partition layout and near the 64 KB descriptor-size cap.
- each block store is split 127+1 partitions: the HWDGE round-robins
  descriptors over the 16 SDMA engines starting at engine 0, and engine 15
  also hosts the dynamic-queue rings, making it ~20% slower than the rest;
  a 127-descriptor DMA gives it 7 descriptors instead of 8, rebalancing
  the tail (the 1-partition remainder DMA lands on fast engine 0).
- the overlapping tail block (workers 1744..1999; rows 1744..1791 stored
  twice with identical data to keep 2000 = 7*256 + 208 full-width) runs
  FIRST, in column halves, so the store stream starts as soon as one
  half-ACT finishes and the write-after-write wait of block 6 on the
  overlap resolves long before block 6 issues.
- ScalarE only ever runs Softplus (prologue) and Exp (main), so there are
  exactly two ACT table loads, both overlapped with the input DMAs.
"""

import numpy as np

WORKERS = 2000
TASKS = 5000
ET = 10
AB = 64
NCORES = 8
F = TASKS * ET  # 50000 output cols
FS = F // NCORES  # 6250 cols per core
FH = FS // 2
LN9 = float(np.log(9.0))

NBLK = 7  # aligned 256-row blocks; tail block overlaps at 1744
TAIL0 = WORKERS - 256  # 1744

_CACHE = {}


def _build_nc():
    import concourse.bass as bass
    import concourse.mybir as mybir
    from concourse import bacc
    from concourse.tile import TileContext
    from contextlib import ExitStack

    f32 = mybir.dt.float32
    AF = mybir.ActivationFunctionType
    OP = mybir.AluOpType

    nc = bacc.Bacc("TRN2")
    wk = nc.dram_tensor("wk", [WORKERS, AB], f32, kind="ExternalInput")
    tf0 = nc.dram_tensor("tf0", [1, FS], f32, kind="ExternalInput")
    onesd = nc.dram_tensor("ones", [1, 128], f32, kind="ExternalInput")
    Wd = nc.dram_tensor("W", [AB, 1], f32, kind="ExternalInput")
    bd = nc.dram_tensor("b", [1], f32, kind="ExternalInput")
    out = nc.dram_tensor("out", [WORKERS, FS], f32, kind="ExternalOutput")

    with TileContext(nc) as tc, ExitStack() as ctx:
        const = ctx.enter_context(tc.tile_pool(name="const", bufs=1))
        psum = ctx.enter_context(
            tc.tile_pool(name="ps", bufs=4, space=bass.MemorySpace.PSUM)
        )
        stage_p = ctx.enter_context(tc.tile_pool(name="stagep", bufs=3))

        # ---- input DMAs, in sync-queue order: tau first (1 descriptor, so
        # the PE broadcast starts immediately), then the worker features.
        taub = const.tile([128, FS], f32, name="taub")
        nc.sync.dma_start(out=taub[0:1, :], in_=tf0[:])
        ones_t = const.tile([1, 128], f32, name="ones")
        nc.sync.dma_start(out=ones_t, in_=onesd[:])
        # worker features, two workers per (partition, block) as 512B
        # descriptors: wka[p, g, (c a)] = wk[256g + 2p + c, a], g=7 = tail
        wka = const.tile([128, 8, 2 * AB], f32, name="wka")
        nc.sync.dma_start(
            out=wka[:, 0:NBLK, :],
            in_=wk[0 : 256 * NBLK, :].rearrange("(g p c) a -> p g (c a)", p=128, c=2),
        )
        nc.sync.dma_start(
            out=wka[:, NBLK : NBLK + 1, :],
            in_=wk[TAIL0:WORKERS, :].rearrange("(g p c) a -> p g (c a)", g=1, c=2),
        )
        Wb = const.tile([128, AB], f32, name="Wb")
        nc.sync.dma_start(
            out=Wb, in_=Wd[:].rearrange("a b -> b a").to_broadcast((128, AB))
        )
        bcol = const.tile([128, 1], f32, name="bcol")
        nc.sync.dma_start(out=bcol, in_=bd[:].to_broadcast((128, 1)))

        # ---- PE partition-broadcast of tau: taub[p, n] = tau[n] for all p.
        # ones[1,128]^T @ tau[1, chunk] -> PSUM [128, chunk]; DVE copies out.
        CH = 512  # one PSUM bank of f32
        chunks = []
        for n0 in range(0, FS, CH):
            n1 = min(n0 + CH, FS)
            ps = psum.tile([128, CH], f32, name="ps", tag="ps")
            nc.tensor.matmul(
                ps[:, 0 : n1 - n0],
                ones_t[:],
                taub[0:1, n0:n1],
                start=True,
                stop=True,
            )
            chunks.append((n0, n1, ps))
        for n0, n1, ps in chunks:
            nc.vector.tensor_copy(taub[:, n0:n1], ps[:, 0 : n1 - n0])

        # ---- per-worker scalars a (ACT scale) and c (ACT bias).
        # Column j = 2g + c holds worker 256g + 2p + c (g=7: 1744 + 2p + c).
        NJ = 16
        wk16 = wka[:].rearrange("p g (c a) -> p (g c) a", a=AB)
        WbT = bass.AP(
            tensor=Wb.tensor,
            offset=Wb.offset,
            ap=[list(Wb.ap[0]), [0, NJ], [1, AB]],
        )
        prod = const.tile([128, NJ, AB], f32, name="prod")
        nc.vector.tensor_mul(prod, wk16, WbT)
        zb_ = const.tile([128, NJ], f32, name="zb")
        nc.vector.reduce_sum(
            out=zb_.rearrange("p (t o) -> p t o", o=1),
            in_=prod,
            axis=mybir.AxisListType.X,
        )
        a_ = const.tile([128, NJ], f32, name="a")
        nc.vector.tensor_scalar(
            out=a_, in0=zb_, scalar1=bcol, scalar2=LN9, op0=OP.add, op1=OP.add
        )
        # p2 = 1 / (9 * (1 + exp(z + b))), computed exactly with the SAME
        # Exp table as the main loop (no second ACT table load); the main
        # ACT then computes exp(a*tau) and DVE scales by p2 per partition.
        eb_ = const.tile([128, NJ], f32, name="eb")
        nc.scalar.activation(out=eb_, in_=zb_, func=AF.Exp, bias=bcol, scale=1.0)
        den_ = const.tile([128, NJ], f32, name="den")
        nc.vector.tensor_scalar(
            out=den_, in0=eb_, scalar1=1.0, scalar2=9.0, op0=OP.add, op1=OP.mult
        )
        p2_ = const.tile([128, NJ], f32, name="p2")
        nc.vector.reciprocal(out=p2_, in_=den_)

        # ---- tail block first (workers 1744..1999), in column halves so the
        # store stream ramps as soon as the first half-ACT lands.
        stgT = stage_p.tile([128, 2, FS], f32, name="stgT", tag="stg")
        dstT = out[TAIL0:WORKERS, :].rearrange("(p c) f -> p c f", c=2)
        for cpar in (0, 1):
            j = 2 * NBLK + cpar
            for c0, c1 in ((0, FH), (FH, FS)):
                nc.scalar.activation(
                    out=stgT[:, cpar, c0:c1],
                    in_=taub[:, c0:c1],
                    func=AF.Exp,
                    scale=a_[:, j : j + 1],
                )
                nc.vector.tensor_scalar_mul(
                    stgT[:, cpar, c0:c1],
                    stgT[:, cpar, c0:c1],
                    p2_[:, j : j + 1],
                )
                nc.sync.dma_start(
                    out=dstT[:, cpar, c0:c1], in_=stgT[:, cpar, c0:c1]
                )

        # ---- aligned blocks: two full-width ACTs, then one DMA whose
        # per-partition descriptor is 50 KB contiguous (workers 2p, 2p+1),
        # split 127+1 to shift one descriptor per block off slow engine 15.
        for g in range(NBLK):
            stg = stage_p.tile([128, 2, FS], f32, name="stg", tag="stg")
            for cpar in (0, 1):
                j = 2 * g + cpar
                nc.scalar.activation(
                    out=stg[:, cpar, :],
                    in_=taub,
                    func=AF.Exp,
                    scale=a_[:, j : j + 1],
                )
                nc.vector.tensor_scalar_mul(
                    stg[:, cpar, :],
                    stg[:, cpar, :],
                    p2_[:, j : j + 1],
                )
            w0 = 256 * g
            dst = out[w0 : w0 + 254, :].rearrange("(p c) f -> p c f", c=2)
            nc.sync.dma_start(out=dst, in_=stg[0:127, :, :])
            dst1 = out[w0 + 254 : w0 + 256, :].rearrange(
                "(p c) f -> p c f", p=1, c=2
            )
            nc.sync.dma_start(out=dst1, in_=stg[127:128, :, :])

    nc.compile()
    return nc


def _get_nc():
    if "nc" not in _CACHE:
        _CACHE["nc"] = _build_nc()
    return _CACHE["nc"]


def _make_in_maps(inputs_arr, W, b):
    wk = np.ascontiguousarray(inputs_arr[:WORKERS, :AB], dtype=np.float32)
    tau_flat = np.ascontiguousarray(
        inputs_arr[WORKERS:, :ET], dtype=np.float32
    ).reshape(F)
    W = np.ascontiguousarray(W, dtype=np.float32)
    b = np.ascontiguousarray(b, dtype=np.float32)
    ones = np.ones((1, 128), dtype=np.float32)
    maps = []
    for c in range(NCORES):
        tf0 = np.ascontiguousarray(tau_flat[c * FS : (c + 1) * FS]).reshape(1, FS)
        maps.append({"wk": wk, "tf0": tf0, "ones": ones, "W": W, "b": b})
    return maps


def _run(inputs_arr, W, b, **kwargs):
    from concourse import bass_utils

    nc = _get_nc()
    in_maps = _make_in_maps(inputs_arr, W, b)
    return bass_utils.run_bass_kernel_spmd(
        nc, in_maps, core_ids=list(range(NCORES)), **kwargs
    )


def kernel(inputs, W, b):
    inputs_arr = np.asarray(inputs, dtype=np.float32)
    last_err = None
    for _ in range(3):  # retry transient device failures
        try:
            res = _run(inputs_arr, np.asarray(W), np.asarray(b))
            break
        except Exception as e:  # noqa: BLE001
            last_err = e
    else:
        raise last_err
    out = np.concatenate([r["out"] for r in res.results], axis=1)
    return out.reshape(WORKERS, TASKS, ET)


# revision 8
# speedup vs baseline: 8.4562x; 8.4562x over previous
"""Trainium2 Bass kernel for nn_Decoder_36636071035490.

Reference computes, for workers i and task/edge (j,l):
    z = worker_feature @ W            # [2000, 1]
    p1 = sigmoid(z + b)
    p2 = (1 - p1) / 9
    P[i, j, l] = p1_i^tau_jl * p2_i^(1 - tau_jl)      # [2000, 5000, 10] f32

Identity used on device (exact in exact arithmetic):
    a_i  = (z_i + b) + ln 9           # ln(p1/p2), since logit(sigmoid(x)) = x
    p2_i = 1 / (9 * (1 + exp(z_i + b)))
    P[i, f] = p2_i * exp(a_i * tau_f)

Sharding: by output columns (task*edge flattened, 50000 -> 8 x 6250); every
core computes the cheap per-worker scalars a/p2 for all 2000 workers
(replicated matvec) and produces the full-height [2000, 6250] slab of P.

Device-side layout (tuned against SDMA-engine traces):
- tau is loaded ONCE as [1, 6250] (25 KB) and broadcast across the 128 SBUF
  partitions by the idle PE (ones[1,128]^T @ tau chunk -> PSUM bank, DVE
  copies PSUM->SBUF), replacing a 3.2 MB replicated HBM read.
- ScalarE runs ONLY Exp (one ACT table load, overlapped with input DMAs):
  the per-worker ln(p2) bias would need an Ln/Softplus table switch, so
  instead ACT produces exp(a*tau) and the mostly-idle DVE multiplies by the
  per-partition scalar p2 in place before the store.
- workers are processed in blocks interleaved two-per-partition (partition
  p of a block holds workers off+2p and off+2p+1), so a block's store is
  one DMA whose per-partition descriptor is a CONTIGUOUS 50 KB range of the
  output (near the 64 KB descriptor cap, 26.4 GB/s/engine).
- HWDGE splits a store across (largest divisor of partition-count <= 16)
  SDMA engines, in partition order starting at engine 0 (measured).  SDMA
  engine 15 also hosts the dynamic-queue rings and runs ~20% slower, so the
  2000 workers are split 5 blocks x 256 (128 partitions -> all 16 engines)
  + 3 blocks x 240 (120 partitions -> engines 0..14 only, engine 15 idle):
  engine 15 moves 2.0 MB while engines 0-14 move 3.2 MB, which matches
  their measured speed ratio.  Exact cover - no overlap rows, no double
  stores.
- block 0 is ACTed/stored in column quarters and block 1 in halves so the
  store stream starts as soon as the first quarter-ACT (and first PE
  broadcast chunks) land; remaining blocks go full-width.
"""

import numpy as np

WORKERS = 2000
TASKS = 5000
ET = 10
AB = 64
NCORES = 8
F = TASKS * ET  # 50000 output cols
FS = F // NCORES  # 6250 cols per core
LN9 = float(np.log(9.0))

# 5 blocks of 256 workers (128 partitions) + 3 blocks of 240 (120 partitions)
BLOCKS = [(g * 256, 128) for g in range(5)] + [
    (1280 + g * 240, 120) for g in range(3)
]
# column split points per block: quarters for block 0, halves for block 1
_Q = [0, 1563, 3126, 4688, FS]
_H = [0, FS // 2, FS]
_SPLITS = [_Q, _H] + [[0, FS]] * 6

_CACHE = {}


def _build_nc():
    import concourse.bass as bass
    import concourse.mybir as mybir
    from concourse import bacc
    from concourse.tile import TileContext
    from contextlib import ExitStack

    f32 = mybir.dt.float32
    AF = mybir.ActivationFunctionType
    OP = mybir.AluOpType

    nc = bacc.Bacc("TRN2")
    wk = nc.dram_tensor("wk", [WORKERS, AB], f32, kind="ExternalInput")
    tf0 = nc.dram_tensor("tf0", [1, FS], f32, kind="ExternalInput")
    onesd = nc.dram_tensor("ones", [1, 128], f32, kind="ExternalInput")
    Wd = nc.dram_tensor("W", [AB, 1], f32, kind="ExternalInput")
    bd = nc.dram_tensor("b", [1], f32, kind="ExternalInput")
    out = nc.dram_tensor("out", [WORKERS, FS], f32, kind="ExternalOutput")

    with TileContext(nc) as tc, ExitStack() as ctx:
        const = ctx.enter_context(tc.tile_pool(name="const", bufs=1))
        psum = ctx.enter_context(
            tc.tile_pool(name="ps", bufs=4, space=bass.MemorySpace.PSUM)
        )
        stage_p = ctx.enter_context(tc.tile_pool(name="stagep", bufs=3))

        # ---- input DMAs, in sync-queue order: tau first (smallest, unblocks
        # the PE broadcast), then worker features.
        taub = const.tile([128, FS], f32, name="taub")
        nc.sync.dma_start(out=taub[0:1, :], in_=tf0[:])
        ones_t = const.tile([1, 128], f32, name="ones")
        nc.sync.dma_start(out=ones_t, in_=onesd[:])
        # worker features, two workers per (partition, block) as 512B
        # descriptors: wka[p, g, (c a)] = wk[off_g + 2p + c, a]
        wka = const.tile([128, 8, 2 * AB], f32, name="wka")
        nc.sync.dma_start(
            out=wka[:, 0:5, :],
            in_=wk[0:1280, :].rearrange("(g p c) a -> p g (c a)", p=128, c=2),
        )
        nc.sync.dma_start(
            out=wka[0:120, 5:8, :],
            in_=wk[1280:2000, :].rearrange("(g p c) a -> p g (c a)", p=120, c=2),
        )
        # pad the unused lanes of the 120-partition blocks with valid data
        # (any rows) so the prologue never reads uninitialized SBUF
        nc.sync.dma_start(
            out=wka[120:128, 5:8, :],
            in_=wk[0:48, :].rearrange("(g p c) a -> p g (c a)", p=8, c=2),
        )
        Wb = const.tile([128, AB], f32, name="Wb")
        nc.sync.dma_start(
            out=Wb, in_=Wd[:].rearrange("a b -> b a").to_broadcast((128, AB))
        )
        bcol = const.tile([128, 1], f32, name="bcol")
        nc.sync.dma_start(out=bcol, in_=bd[:].to_broadcast((128, 1)))

        # ---- PE partition-broadcast of tau: taub[p, n] = tau[n] for all p.
        CH = 512  # one PSUM bank of f32
        chunks = []
        for n0 in range(0, FS, CH):
            n1 = min(n0 + CH, FS)
            ps = psum.tile([128, CH], f32, name="ps", tag="ps")
            nc.tensor.matmul(
                ps[:, 0 : n1 - n0],
                ones_t[:],
                taub[0:1, n0:n1],
                start=True,
                stop=True,
            )
            chunks.append((n0, n1, ps))
        for n0, n1, ps in chunks:
            nc.vector.tensor_copy(taub[:, n0:n1], ps[:, 0 : n1 - n0])

        # ---- per-worker scalars a (ACT scale) and p2 (DVE post-multiply).
        # Column j = 2g + c holds worker off_g + 2p + c.
        NJ = 16
        wk16 = wka[:].rearrange("p g (c a) -> p (g c) a", a=AB)
        WbT = bass.AP(
            tensor=Wb.tensor,
            offset=Wb.offset,
            ap=[list(Wb.ap[0]), [0, NJ], [1, AB]],
        )
        prod = const.tile([128, NJ, AB], f32, name="prod")
        nc.vector.tensor_mul(prod, wk16, WbT)
        zb_ = const.tile([128, NJ], f32, name="zb")
        nc.vector.reduce_sum(
            out=zb_.rearrange("p (t o) -> p t o", o=1),
            in_=prod,
            axis=mybir.AxisListType.X,
        )
        a_ = const.tile([128, NJ], f32, name="a")
        nc.vector.tensor_scalar(
            out=a_, in0=zb_, scalar1=bcol, scalar2=LN9, op0=OP.add, op1=OP.add
        )
        eb_ = const.tile([128, NJ], f32, name="eb")
        nc.scalar.activation(out=eb_, in_=zb_, func=AF.Exp, bias=bcol, scale=1.0)
        den_ = const.tile([128, NJ], f32, name="den")
        nc.vector.tensor_scalar(
            out=den_, in0=eb_, scalar1=1.0, scalar2=9.0, op0=OP.add, op1=OP.mult
        )
        p2_ = const.tile([128, NJ], f32, name="p2")
        nc.vector.reciprocal(out=p2_, in_=den_)

        # ---- main loop: ACT exp(a*tau) -> DVE *= p2 -> store.
        for g, (off, P) in enumerate(BLOCKS):
            stg = stage_p.tile([128, 2, FS], f32, name="stg", tag="stg")
            dst = out[off : off + 2 * P, :].rearrange("(p c) f -> p c f", c=2)
            sp = _SPLITS[g]
            for cpar in (0, 1):
                j = 2 * g + cpar
                for c0, c1 in zip(sp[:-1], sp[1:]):
                    nc.scalar.activation(
                        out=stg[0:P, cpar, c0:c1],
                        in_=taub[0:P, c0:c1],
                        func=AF.Exp,
                        scale=a_[0:P, j : j + 1],
                    )
                    nc.vector.tensor_scalar_mul(
                        stg[0:P, cpar, c0:c1],
                        stg[0:P, cpar, c0:c1],
                        p2_[0:P, j : j + 1],
                    )
                    if len(sp) > 2:
                        nc.sync.dma_start(
                            out=dst[:, cpar, c0:c1], in_=stg[0:P, cpar, c0:c1]
                        )
            if len(sp) == 2:
                nc.sync.dma_start(out=dst, in_=stg[0:P, :, :])

    nc.compile()
    return nc


def _get_nc():
    if "nc" not in _CACHE:
        _CACHE["nc"] = _build_nc()
    return _CACHE["nc"]


def _make_in_maps(inputs_arr, W, b):
    wk = np.ascontiguousarray(inputs_arr[:WORKERS, :AB], dtype=np.float32)
    tau_flat = np.ascontiguousarray(
        inputs_arr[WORKERS:, :ET], dtype=np.float32
    ).reshape(F)
    W = np.ascontiguousarray(W, dtype=np.float32)
    b = np.ascontiguousarray(b, dtype=np.float32)
    ones = np.ones((1, 128), dtype=np.float32)
    maps = []
    for c in range(NCORES):
        tf0 = np.ascontiguousarray(tau_flat[c * FS : (c + 1) * FS]).reshape(1, FS)
        maps.append({"wk": wk, "tf0": tf0, "ones": ones, "W": W, "b": b})
    return maps


def _run(inputs_arr, W, b, **kwargs):
    from concourse import bass_utils

    nc = _get_nc()
    in_maps = _make_in_maps(inputs_arr, W, b)
    return bass_utils.run_bass_kernel_spmd(
        nc, in_maps, core_ids=list(range(NCORES)), **kwargs
    )


def kernel(inputs, W, b):
    inputs_arr = np.asarray(inputs, dtype=np.float32)
    last_err = None
    for _ in range(3):  # retry transient device failures
        try:
            res = _run(inputs_arr, np.asarray(W), np.asarray(b))
            break
        except Exception as e:  # noqa: BLE001
            last_err = e
    else:
        raise last_err
    out = np.concatenate([r["out"] for r in res.results], axis=1)
    return out.reshape(WORKERS, TASKS, ET)


# revision 9
# speedup vs baseline: 8.5420x; 1.0101x over previous
"""Trainium2 Bass kernel for nn_Decoder_36636071035490.

Reference computes, for workers i and task/edge (j,l):
    z = worker_feature @ W            # [2000, 1]
    p1 = sigmoid(z + b)
    p2 = (1 - p1) / 9
    P[i, j, l] = p1_i^tau_jl * p2_i^(1 - tau_jl)      # [2000, 5000, 10] f32

Identities used on device (exact in exact arithmetic):
    a_i  = (z_i + b) + ln 9           # ln(p1/p2), since logit(sigmoid(x)) = x
    p2_i = 1 / (9 * (1 + exp(z_i + b)))
    c_i  = ln(p2_i) = -ln(1 + exp(z_i + b)) - ln 9
    P[i, f] = exp(a_i * tau_f + c_i)  = p2_i * exp(a_i * tau_f)

Sharding: by output columns (task*edge flattened, 50000 -> 8 x 6250); every
core computes the cheap per-worker scalars for all 2000 workers (replicated
matvec) and produces the full-height [2000, 6250] slab of P.

Device-side schedule (tuned against SDMA-engine traces):
- workers are processed in blocks interleaved two-per-partition (partition
  p of a block holds workers off+2p and off+2p+1), so a block's store is
  one DMA whose per-partition descriptor is a CONTIGUOUS 50 KB range of the
  output (near the 64 KB descriptor cap; 26.4 GB/s per SDMA engine).
- HWDGE splits a store across (largest divisor of partition-count <= 16)
  SDMA engines in partition order starting at engine 0 (measured), and SDMA
  engine 15 also hosts the dynamic-queue rings making it ~20% slower, so
  the 2000 workers are split 5 blocks x 256 (128 partitions -> all 16
  engines) + 3 blocks x 240 (120 partitions -> engines 0..14, engine 15
  idle): engine 15 moves 2 MB vs 3.2 MB on engines 0-14, matching their
  speed ratio.  Exact cover: no overlap rows, no double stores.
- tau columns [0:2048] come from a stride-0 broadcast DMA (lands with the
  input loads), the rest from the idle PE (ones[1,128]^T @ tau chunk ->
  PSUM, DVE copies out) - fp32 PE matmuls are 2-pass so this chain takes
  ~17 us, which is why the early columns bypass it.
- block 0 ramps with scale-only Exp quarters multiplied by p2 on the DVE
  (no dependence on the Ln table), storing each quarter immediately.  The
  Ln chain for the bias c (used by all later blocks) runs BETWEEN block-0
  quarters, where ScalarE would be stalled on the PE chain anyway.  Blocks
  1-7 then run pure exp(a*tau+c) ACTs with NO steady-state DVE traffic
  (a DVE post-multiply on every block measurably degrades concurrent SDMA
  store throughput ~25%).
"""

import numpy as np

WORKERS = 2000
TASKS = 5000
ET = 10
AB = 64
NCORES = 8
F = TASKS * ET  # 50000 output cols
FS = F // NCORES  # 6250 cols per core
LN9 = float(np.log(9.0))

# 5 blocks of 256 workers (128 partitions) + 3 blocks of 240 (120 partitions)
BLOCKS = [(g * 256, 128) for g in range(5)] + [
    (1280 + g * 240, 120) for g in range(3)
]
TB = 2048  # tau columns broadcast by DMA; the rest go through the PE
# column split points: quarters for block 0, halves for block 1, then full
_Q = [0, 1563, 3126, 4688, FS]
_H = [0, FS // 2, FS]
_SPLITS = [_Q, _H] + [[0, FS]] * 6

_CACHE = {}


def _build_nc():
    import concourse.bass as bass
    import concourse.mybir as mybir
    from concourse import bacc
    from concourse.tile import TileContext
    from contextlib import ExitStack

    f32 = mybir.dt.float32
    AF = mybir.ActivationFunctionType
    OP = mybir.AluOpType

    nc = bacc.Bacc("TRN2")
    wk = nc.dram_tensor("wk", [WORKERS, AB], f32, kind="ExternalInput")
    tf0 = nc.dram_tensor("tf0", [1, FS], f32, kind="ExternalInput")
    onesd = nc.dram_tensor("ones", [1, 128], f32, kind="ExternalInput")
    Wd = nc.dram_tensor("W", [AB, 1], f32, kind="ExternalInput")
    bd = nc.dram_tensor("b", [1], f32, kind="ExternalInput")
    out = nc.dram_tensor("out", [WORKERS, FS], f32, kind="ExternalOutput")

    with TileContext(nc) as tc, ExitStack() as ctx:
        const = ctx.enter_context(tc.tile_pool(name="const", bufs=1))
        psum = ctx.enter_context(
            tc.tile_pool(name="ps", bufs=4, space=bass.MemorySpace.PSUM)
        )
        stage_p = ctx.enter_context(tc.tile_pool(name="stagep", bufs=3))

        # ---- input DMAs, in sync-queue order.  The tau broadcast for the
        # early columns goes first so block 0's first ACT unblocks ASAP.
        taub = const.tile([128, FS], f32, name="taub")
        nc.sync.dma_start(
            out=taub[:, 0:TB], in_=tf0[0:1, 0:TB].to_broadcast((128, TB))
        )
        nc.sync.dma_start(out=taub[0:1, TB:FS], in_=tf0[0:1, TB:FS])
        ones_t = const.tile([1, 128], f32, name="ones")
        nc.sync.dma_start(out=ones_t, in_=onesd[:])
        # worker features, two workers per (partition, block) as 512B
        # descriptors: wka[p, g, (c a)] = wk[off_g + 2p + c, a]
        wka = const.tile([128, 8, 2 * AB], f32, name="wka")
        nc.sync.dma_start(
            out=wka[:, 0:5, :],
            in_=wk[0:1280, :].rearrange("(g p c) a -> p g (c a)", p=128, c=2),
        )
        nc.sync.dma_start(
            out=wka[0:120, 5:8, :],
            in_=wk[1280:2000, :].rearrange("(g p c) a -> p g (c a)", p=120, c=2),
        )
        # pad the unused lanes of the 120-partition blocks with valid data
        # (any rows) so the prologue never reads uninitialized SBUF
        nc.sync.dma_start(
            out=wka[120:128, 5:8, :],
            in_=wk[0:48, :].rearrange("(g p c) a -> p g (c a)", p=8, c=2),
        )
        Wb = const.tile([128, AB], f32, name="Wb")
        nc.sync.dma_start(
            out=Wb, in_=Wd[:].rearrange("a b -> b a").to_broadcast((128, AB))
        )
        bcol = const.tile([128, 1], f32, name="bcol")
        nc.sync.dma_start(out=bcol, in_=bd[:].to_broadcast((128, 1)))

        # ---- PE partition-broadcast of tau[TB:FS]
        CH = 512  # one PSUM bank of f32
        chunks = []
        for n0 in range(TB, FS, CH):
            n1 = min(n0 + CH, FS)
            ps = psum.tile([128, CH], f32, name="ps", tag="ps")
            nc.tensor.matmul(
                ps[:, 0 : n1 - n0],
                ones_t[:],
                taub[0:1, n0:n1],
                start=True,
                stop=True,
            )
            chunks.append((n0, n1, ps))
        for n0, n1, ps in chunks:
            nc.vector.tensor_copy(taub[:, n0:n1], ps[:, 0 : n1 - n0])

        # ---- per-worker scalars: a (ACT scale), p2 (block-0 DVE multiply),
        # c = ln p2 (ACT bias for blocks 1+).  Column j = 2g + c_par holds
        # worker off_g + 2p + c_par.
        NJ = 16
        wk16 = wka[:].rearrange("p g (c a) -> p (g c) a", a=AB)
        WbT = bass.AP(
            tensor=Wb.tensor,
            offset=Wb.offset,
            ap=[list(Wb.ap[0]), [0, NJ], [1, AB]],
        )
        prod = const.tile([128, NJ, AB], f32, name="prod")
        nc.vector.tensor_mul(prod, wk16, WbT)
        zb_ = const.tile([128, NJ], f32, name="zb")
        nc.vector.reduce_sum(
            out=zb_.rearrange("p (t o) -> p t o", o=1),
            in_=prod,
            axis=mybir.AxisListType.X,
        )
        a_ = const.tile([128, NJ], f32, name="a")
        nc.vector.tensor_scalar(
            out=a_, in0=zb_, scalar1=bcol, scalar2=LN9, op0=OP.add, op1=OP.add
        )
        eb_ = const.tile([128, NJ], f32, name="eb")
        nc.scalar.activation(out=eb_, in_=zb_, func=AF.Exp, bias=bcol, scale=1.0)
        den_ = const.tile([128, NJ], f32, name="den")
        nc.vector.tensor_scalar(
            out=den_, in0=eb_, scalar1=1.0, scalar2=9.0, op0=OP.add, op1=OP.mult
        )
        p2_ = const.tile([128, NJ], f32, name="p2")
        nc.vector.reciprocal(out=p2_, in_=den_)

        # ---- block 0 ramp: scale-only Exp quarters, DVE multiplies by p2.
        # The Ln chain for c_ is emitted between quarters 2 and 3, where
        # ScalarE stalls on the PE broadcast chain anyway.
        g0_off, g0_P = BLOCKS[0]
        stg0 = stage_p.tile([128, 2, FS], f32, name="stg0", tag="stg")
        dst0 = out[g0_off : g0_off + 2 * g0_P, :].rearrange("(p c) f -> p c f", c=2)
        lb_ = const.tile([128, NJ], f32, name="lb")
        c_ = const.tile([128, NJ], f32, name="c")

        def _b0_quarter(qi):
            c0, c1 = _Q[qi], _Q[qi + 1]
            for cpar in (0, 1):
                j = cpar
                nc.scalar.activation(
                    out=stg0[:, cpar, c0:c1],
                    in_=taub[:, c0:c1],
                    func=AF.Exp,
                    scale=a_[:, j : j + 1],
                )
                nc.vector.tensor_scalar_mul(
                    stg0[:, cpar, c0:c1],
                    stg0[:, cpar, c0:c1],
                    p2_[:, j : j + 1],
                )
                nc.sync.dma_start(
                    out=dst0[:, cpar, c0:c1], in_=stg0[:, cpar, c0:c1]
                )

        _b0_quarter(0)
        _b0_quarter(1)
        # ln(1 + e^(z+b)) via Ln(eb_ + 1); c = -ln(..) - ln 9
        nc.scalar.activation(out=lb_, in_=eb_, func=AF.Ln, bias=1.0, scale=1.0)
        nc.vector.tensor_scalar(
            out=c_, in0=lb_, scalar1=-1.0, scalar2=-LN9, op0=OP.mult, op1=OP.add
        )
        _b0_quarter(2)
        _b0_quarter(3)

        # ---- blocks 1-7: pure exp(a*tau + c) ACTs, no DVE traffic.
        for g in range(1, 8):
            off, P = BLOCKS[g]
            stg = stage_p.tile([128, 2, FS], f32, name="stg", tag="stg")
            dst = out[off : off + 2 * P, :].rearrange("(p c) f -> p c f", c=2)
            sp = _SPLITS[g]
            for cpar in (0, 1):
                j = 2 * g + cpar
                for c0, c1 in zip(sp[:-1], sp[1:]):
                    nc.scalar.activation(
                        out=stg[0:P, cpar, c0:c1],
                        in_=taub[0:P, c0:c1],
                        func=AF.Exp,
                        bias=c_[0:P, j : j + 1],
                        scale=a_[0:P, j : j + 1],
                    )
                    if len(sp) > 2:
                        nc.sync.dma_start(
                            out=dst[:, cpar, c0:c1], in_=stg[0:P, cpar, c0:c1]
                        )
            if len(sp) == 2:
                nc.sync.dma_start(out=dst, in_=stg[0:P, :, :])

    nc.compile()
    return nc


def _get_nc():
    if "nc" not in _CACHE:
        _CACHE["nc"] = _build_nc()
    return _CACHE["nc"]


def _make_in_maps(inputs_arr, W, b):
    wk = np.ascontiguousarray(inputs_arr[:WORKERS, :AB], dtype=np.float32)
    tau_flat = np.ascontiguousarray(
        inputs_arr[WORKERS:, :ET], dtype=np.float32
    ).reshape(F)
    W = np.ascontiguousarray(W, dtype=np.float32)
    b = np.ascontiguousarray(b, dtype=np.float32)
    ones = np.ones((1, 128), dtype=np.float32)
    maps = []
    for c in range(NCORES):
        tf0 = np.ascontiguousarray(tau_flat[c * FS : (c + 1) * FS]).reshape(1, FS)
        maps.append({"wk": wk, "tf0": tf0, "ones": ones, "W": W, "b": b})
    return maps


def _run(inputs_arr, W, b, **kwargs):
    from concourse import bass_utils

    nc = _get_nc()
    in_maps = _make_in_maps(inputs_arr, W, b)
    return bass_utils.run_bass_kernel_spmd(
        nc, in_maps, core_ids=list(range(NCORES)), **kwargs
    )


def kernel(inputs, W, b):
    inputs_arr = np.asarray(inputs, dtype=np.float32)
    last_err = None
    for _ in range(3):  # retry transient device failures
        try:
            res = _run(inputs_arr, np.asarray(W), np.asarray(b))
            break
        except Exception as e:  # noqa: BLE001
            last_err = e
    else:
        raise last_err
    out = np.concatenate([r["out"] for r in res.results], axis=1)
    return out.reshape(WORKERS, TASKS, ET)


# revision 10
# speedup vs baseline: 9.7433x; 1.1406x over previous
"""Trainium2 Bass kernel for nn_Decoder_36636071035490.

Reference computes, for workers i and task/edge (j,l):
    z = worker_feature @ W            # [2000, 1]
    p1 = sigmoid(z + b)
    p2 = (1 - p1) / 9
    P[i, j, l] = p1_i^tau_jl * p2_i^(1 - tau_jl)      # [2000, 5000, 10] f32

Identities used on device (exact in exact arithmetic):
    a_i  = (z_i + b) + ln 9           # ln(p1/p2), since logit(sigmoid(x)) = x
    p2_i = 1 / (9 * (1 + exp(z_i + b)))
    c_i  = ln(p2_i) = -ln(1 + exp(z_i + b)) - ln 9
    P[i, f] = exp(a_i * tau_f + c_i)  = p2_i * exp(a_i * tau_f)

Sharding: by output columns (task*edge flattened, 50000 -> 8 x 6250); every
core computes the cheap per-worker scalars for all 2000 workers (replicated
matvec) and produces the full-height [2000, 6250] slab of P.

Device-side schedule (tuned against SDMA-engine traces):
- workers are processed in blocks interleaved two-per-partition (partition
  p of a block holds workers off+2p and off+2p+1), so a block's store is
  one DMA whose per-partition descriptor is a CONTIGUOUS 50 KB range of the
  output (near the 64 KB cap).  128-partition stores get the port-aligned
  16-engine split and run 26.7 GB/s per SDMA engine; any other partition
  count falls back to a consecutive split at roughly HALF that rate on
  (largest divisor <= 16) engines - measured.
- SDMA engine 15 also serves the dynamic-queue rings and only manages
  ~21 GB/s, so it must carry fewer bytes than engines 0-14.  The only
  full-rate stores are 128-partition ones (which load all 16 engines
  evenly), so the skew comes from ONE half-rate block: 7 blocks x 256
  workers (128 partitions, all 16 engines) + 1 block x 208 workers (104
  partitions -> engines 0..12 only, engine 15 idle).  Exact cover of 2000
  workers, no double stores.  Engine 15 ends at ~2.8 MB vs ~3.2+ MB for
  engines 0-12, matching the measured speed gap.
- tau columns [0:2048] come from a stride-0 broadcast DMA; the rest are
  broadcast by the idle PE (ones[1,128]^T @ tau chunk -> PSUM bank, DVE
  copies PSUM->SBUF).  fp32 PE matmuls are 2-pass, so this chain takes
  ~17 us - the DMA part covers the early ACTs while it runs.
- block 0 ramps with scale-only Exp quarters multiplied by p2 on the DVE
  (no dependence on the Ln table); each quarter is stored immediately.
  The Ln chain producing the bias c runs between block-0 quarters where
  ScalarE would idle anyway.  Blocks 1+ run pure exp(a*tau+c) ACTs with no
  steady-state DVE traffic (a per-block DVE post-multiply measurably
  degrades concurrent SDMA store throughput ~25%).
- the tiny prologue (matvec z, a, p2) is emitted BEFORE the PSUM copy
  chain so the tile scheduler runs it the moment the worker features land
  instead of behind 13 PSUM copies.
"""

import numpy as np

WORKERS = 2000
TASKS = 5000
ET = 10
AB = 64
NCORES = 8
F = TASKS * ET  # 50000 output cols
FS = F // NCORES  # 6250 cols per core
LN9 = float(np.log(9.0))

# 7 blocks of 256 workers (128 partitions, full-rate stores) + 1 block of
# 208 workers (104 partitions -> engines 0..12, skews work off engine 15)
BLOCKS = [(g * 256, 128) for g in range(7)] + [(1792, 104)]
# processing order: ramp block, one more 128p block, then the slow block
# early (its packets sit in the per-engine FIFOs; order doesn't change
# engine finish times but keeps the kernel tail all full-rate)
ORDER = [0, 1, 7, 2, 3, 4, 5, 6]
TB = 2048  # tau columns broadcast by DMA; the rest go through the PE
_Q = [0, 1563, 3126, 4688, FS]  # block-0 ramp quarters
_H = [0, FS // 2, FS]  # block-1 halves

_CACHE = {}


def _build_nc():
    import concourse.bass as bass
    import concourse.mybir as mybir
    from concourse import bacc
    from concourse.tile import TileContext
    from contextlib import ExitStack

    f32 = mybir.dt.float32
    AF = mybir.ActivationFunctionType
    OP = mybir.AluOpType

    nc = bacc.Bacc("TRN2")
    wk = nc.dram_tensor("wk", [WORKERS, AB], f32, kind="ExternalInput")
    tf0 = nc.dram_tensor("tf0", [1, FS], f32, kind="ExternalInput")
    onesd = nc.dram_tensor("ones", [1, 128], f32, kind="ExternalInput")
    Wd = nc.dram_tensor("W", [AB, 1], f32, kind="ExternalInput")
    bd = nc.dram_tensor("b", [1], f32, kind="ExternalInput")
    out = nc.dram_tensor("out", [WORKERS, FS], f32, kind="ExternalOutput")

    with TileContext(nc) as tc, ExitStack() as ctx:
        const = ctx.enter_context(tc.tile_pool(name="const", bufs=1))
        psum = ctx.enter_context(
            tc.tile_pool(name="ps", bufs=4, space=bass.MemorySpace.PSUM)
        )
        stage_p = ctx.enter_context(tc.tile_pool(name="stagep", bufs=3))

        # ---- input DMAs.  Order matters: the sync queue is FIFO and the
        # PE chain + prologue are the ramp critical path, so the tiny loads
        # they need go first, then the worker features, then the 1 MB tau
        # broadcast, which only gates the very first ACT quarter.
        taub = const.tile([128, FS], f32, name="taub")
        ones_t = const.tile([1, 128], f32, name="ones")
        wka = const.tile([128, 8, 2 * AB], f32, name="wka")
        Wb = const.tile([128, AB], f32, name="Wb")
        bcol = const.tile([128, 1], f32, name="bcol")

        nc.sync.dma_start(out=taub[0:1, TB:FS], in_=tf0[0:1, TB:FS])
        nc.sync.dma_start(out=ones_t, in_=onesd[:])
        nc.sync.dma_start(out=bcol, in_=bd[:].to_broadcast((128, 1)))
        nc.sync.dma_start(
            out=Wb, in_=Wd[:].rearrange("a b -> b a").to_broadcast((128, AB))
        )
        # worker features, two workers per (partition, block) as 512B
        # descriptors: wka[p, g, (c a)] = wk[off_g + 2p + c, a]
        nc.sync.dma_start(
            out=wka[:, 0:7, :],
            in_=wk[0:1792, :].rearrange("(g p c) a -> p g (c a)", p=128, c=2),
        )
        nc.sync.dma_start(
            out=wka[0:104, 7:8, :],
            in_=wk[1792:2000, :].rearrange("(g p c) a -> p g (c a)", p=104, c=2),
        )
        # pad unused lanes of the 104-partition block with valid data (any
        # rows) so the prologue never reads uninitialized SBUF
        nc.sync.dma_start(
            out=wka[104:128, 7:8, :],
            in_=wk[0:48, :].rearrange("(g p c) a -> p g (c a)", p=24, c=2),
        )
        nc.sync.dma_start(
            out=taub[:, 0:TB], in_=tf0[0:1, 0:TB].to_broadcast((128, TB))
        )

        # ---- per-worker scalars (emitted before the PSUM copies so the
        # scheduler runs them as soon as wka lands).  Column j = 2g + c_par
        # holds worker off_g + 2p + c_par.
        NJ = 16
        wk16 = wka[:].rearrange("p g (c a) -> p (g c) a", a=AB)
        WbT = bass.AP(
            tensor=Wb.tensor,
            offset=Wb.offset,
            ap=[list(Wb.ap[0]), [0, NJ], [1, AB]],
        )
        prod = const.tile([128, NJ, AB], f32, name="prod")
        nc.vector.tensor_mul(prod, wk16, WbT)
        zb_ = const.tile([128, NJ], f32, name="zb")
        nc.vector.reduce_sum(
            out=zb_.rearrange("p (t o) -> p t o", o=1),
            in_=prod,
            axis=mybir.AxisListType.X,
        )
        a_ = const.tile([128, NJ], f32, name="a")
        nc.vector.tensor_scalar(
            out=a_, in0=zb_, scalar1=bcol, scalar2=LN9, op0=OP.add, op1=OP.add
        )
        eb_ = const.tile([128, NJ], f32, name="eb")
        nc.scalar.activation(out=eb_, in_=zb_, func=AF.Exp, bias=bcol, scale=1.0)
        den_ = const.tile([128, NJ], f32, name="den")
        nc.vector.tensor_scalar(
            out=den_, in0=eb_, scalar1=1.0, scalar2=9.0, op0=OP.add, op1=OP.mult
        )
        p2_ = const.tile([128, NJ], f32, name="p2")
        nc.vector.reciprocal(out=p2_, in_=den_)

        # ---- PE partition-broadcast of tau[TB:FS]
        CH = 512  # one PSUM bank of f32
        chunks = []
        for n0 in range(TB, FS, CH):
            n1 = min(n0 + CH, FS)
            ps = psum.tile([128, CH], f32, name="ps", tag="ps")
            nc.tensor.matmul(
                ps[:, 0 : n1 - n0],
                ones_t[:],
                taub[0:1, n0:n1],
                start=True,
                stop=True,
            )
            chunks.append((n0, n1, ps))
        for n0, n1, ps in chunks:
            nc.vector.tensor_copy(taub[:, n0:n1], ps[:, 0 : n1 - n0])

        # ---- block 0 ramp: scale-only Exp quarters, DVE multiplies by p2.
        # The Ln chain producing c_ is emitted between quarters, where
        # ScalarE stalls on the PE broadcast chain anyway.
        stg0 = stage_p.tile([128, 2, FS], f32, name="stg0", tag="stg")
        dst0 = out[0:256, :].rearrange("(p c) f -> p c f", c=2)
        lb_ = const.tile([128, NJ], f32, name="lb")
        c_ = const.tile([128, NJ], f32, name="c")

        def _b0_quarter(qi):
            c0, c1 = _Q[qi], _Q[qi + 1]
            for cpar in (0, 1):
                nc.scalar.activation(
                    out=stg0[:, cpar, c0:c1],
                    in_=taub[:, c0:c1],
                    func=AF.Exp,
                    scale=a_[:, cpar : cpar + 1],
                )
                nc.vector.tensor_scalar_mul(
                    stg0[:, cpar, c0:c1],
                    stg0[:, cpar, c0:c1],
                    p2_[:, cpar : cpar + 1],
                )
                nc.sync.dma_start(
                    out=dst0[:, cpar, c0:c1], in_=stg0[:, cpar, c0:c1]
                )

        _b0_quarter(0)
        _b0_quarter(1)
        # ln(1 + e^(z+b)) via Ln(eb_ + 1); c = -ln(..) - ln 9
        nc.scalar.activation(out=lb_, in_=eb_, func=AF.Ln, bias=1.0, scale=1.0)
        nc.vector.tensor_scalar(
            out=c_, in0=lb_, scalar1=-1.0, scalar2=-LN9, op0=OP.mult, op1=OP.add
        )
        _b0_quarter(2)
        _b0_quarter(3)

        # ---- remaining blocks: pure exp(a*tau + c) ACTs, no DVE traffic.
        for g in ORDER[1:]:
            off, P = BLOCKS[g]
            stg = stage_p.tile([128, 2, FS], f32, name="stg", tag="stg")
            dst = out[off : off + 2 * P, :].rearrange("(p c) f -> p c f", c=2)
            sp = _H if g == 1 else [0, FS]
            for cpar in (0, 1):
                j = 2 * g + cpar
                for c0, c1 in zip(sp[:-1], sp[1:]):
                    nc.scalar.activation(
                        out=stg[0:P, cpar, c0:c1],
                        in_=taub[0:P, c0:c1],
                        func=AF.Exp,
                        bias=c_[0:P, j : j + 1],
                        scale=a_[0:P, j : j + 1],
                    )
                    if len(sp) > 2:
                        nc.sync.dma_start(
                            out=dst[:, cpar, c0:c1], in_=stg[0:P, cpar, c0:c1]
                        )
            if len(sp) == 2:
                nc.sync.dma_start(out=dst, in_=stg[0:P, :, :])

    nc.compile()
    return nc


def _get_nc():
    if "nc" not in _CACHE:
        _CACHE["nc"] = _build_nc()
    return _CACHE["nc"]


def _make_in_maps(inputs_arr, W, b):
    wk = np.ascontiguousarray(inputs_arr[:WORKERS, :AB], dtype=np.float32)
    tau_flat = np.ascontiguousarray(
        inputs_arr[WORKERS:, :ET], dtype=np.float32
    ).reshape(F)
    W = np.ascontiguousarray(W, dtype=np.float32)
    b = np.ascontiguousarray(b, dtype=np.float32)
    ones = np.ones((1, 128), dtype=np.float32)
    maps = []
    for c in range(NCORES):
        tf0 = np.ascontiguousarray(tau_flat[c * FS : (c + 1) * FS]).reshape(1, FS)
        maps.append({"wk": wk, "tf0": tf0, "ones": ones, "W": W, "b": b})
    return maps


def _run(inputs_arr, W, b, **kwargs):
    from concourse import bass_utils

    nc = _get_nc()
    in_maps = _make_in_maps(inputs_arr, W, b)
    return bass_utils.run_bass_kernel_spmd(
        nc, in_maps, core_ids=list(range(NCORES)), **kwargs
    )


def kernel(inputs, W, b):
    inputs_arr = np.asarray(inputs, dtype=np.float32)
    last_err = None
    for _ in range(3):  # retry transient device failures
        try:
            res = _run(inputs_arr, np.asarray(W), np.asarray(b))
            break
        except Exception as e:  # noqa: BLE001
            last_err = e
    else:
        raise last_err
    out = np.concatenate([r["out"] for r in res.results], axis=1)
    return out.reshape(WORKERS, TASKS, ET)
